# revision 34
# baseline (speedup 1.0000x reference)
"""Trainium2 Bass kernel for nn_Adj_adjust (GCN embed + reparameterize + KL).

Strategy (8 NeuronCores, row-sharded over the node dim):
  - Each core c owns rows r0 = c*128 .. r0+128 of adj / noise / outputs.
  - x (as x.T) and W are replicated; each core computes xW = x @ W on PE,
    then h_shard = adj_shard @ xW (row-sharded GEMM).
  - V = noise.mean(axis=1) is a streaming DVE accumulation over the
    52 MB/core noise shard (the memory-bound critical path).
  - x_prob = sigmoid(h), x_sample = sigmoid((V + x_prob - 1)/tau),
    adj_new = adj * x_sample on ACT/DVE.
  - KL partial: ACT Square with accum_out gives per-row sums; host adds
    the 8x128 partials (the "all-reduce" of the sharding hint).

DMA layout: noise chunk loads stream on the Sync HWDGE ring; GEMM
operand loads and all output stores go on the Scalar HWDGE ring so they
never queue behind the 52 MB noise stream.

kernel(**inputs) takes FULL inputs, shards on host (noise/adj shards are
zero-copy row slices), runs one SPMD Bass program on cores 0-7, and
concatenates shard outputs to FULL outputs (h, adj_new, info_loss).
"""

import numpy as np

import concourse.bacc as bacc
import concourse.mybir as mybir
from concourse import tile
from concourse.bass_utils import run_bass_kernel_spmd

N = 1024        # nodes
D = 256         # input feature dim
S = 100         # num_sample
NCORES = 8
RP = N // NCORES  # 128 rows per core

F32 = mybir.dt.float32
AF = mybir.ActivationFunctionType
ALU = mybir.AluOpType

# noise samples per DMA chunk: small first chunk for an early DVE start,
# then 12-sample fp16 chunks (24 KB/partition) so bufs=4 fits in SBUF and
# the DMA trigger queue stays ahead of the stream
CHUNKS = [4] + [12] * 8
F16 = mybir.dt.float16


def build_program(tau: float, threshold: float):
    """Build (and Bacc-compile) the single-core SPMD Bass program."""
    nc = bacc.Bacc("TRN2", target_bir_lowering=False, debug=False,
                   num_devices=NCORES)

    xT_d = nc.dram_tensor("xT", [D, N], F32, kind="ExternalInput")
    W_d = nc.dram_tensor("W", [D, N], F32, kind="ExternalInput")
    adj_d = nc.dram_tensor("adj", [RP, N], F32, kind="ExternalInput")
    adjT_d = nc.dram_tensor("adjT", [N, RP], F32, kind="ExternalInput")
    noise_d = nc.dram_tensor("noise", [RP, S, N], F16, kind="ExternalInput")

    h_d = nc.dram_tensor("h_out", [RP, N], F32, kind="ExternalOutput")
    an_d = nc.dram_tensor("adjnew_out", [RP, N], F32, kind="ExternalOutput")
    lr_d = nc.dram_tensor("loss_out", [RP, 1], F32, kind="ExternalOutput")

    KX = D // 128   # 2 k-tiles for x @ W
    KH = N // 128   # 8 k-tiles for adj @ xW
    NT = N // 512   # 2 n-tiles of 512

    with tile.TileContext(nc) as tc:
        with (
            tc.tile_pool(name="const", bufs=1) as cpool,
            tc.tile_pool(name="noise", bufs=4) as npool,
            tc.tile_pool(name="work", bufs=1) as wpool,
            tc.tile_pool(name="psA", bufs=4, space="PSUM") as psA,
            tc.tile_pool(name="psB", bufs=2, space="PSUM") as psB,
        ):
            # ---- first noise chunk DMA goes out before anything else ----
            # V4 = 4 parallel column accumulators [128, 4, N]; folded at the
            # end. One fp32 tensor_add over [128, 4, N] per half-chunk keeps
            # the DVE op count ~4x lower than per-sample adds.
            V4 = wpool.tile([128, 4, N], F32)
            nz0 = npool.tile([128, CHUNKS[0], N], F16, tag="noise_chunk")
            nc.sync.dma_start(nz0[:], noise_d[:, 0:CHUNKS[0], :])
            nc.gpsimd.memset(V4[:], 0.0)

            # ---- GEMM operands on the Scalar HWDGE ring ----
            xT_t = cpool.tile([128, KX, N], F32)
            nc.scalar.dma_start(xT_t[:], xT_d.rearrange("(k p) n -> p k n", p=128))
            W_t = cpool.tile([128, KX, N], F32)
            nc.scalar.dma_start(W_t[:], W_d.rearrange("(k p) n -> p k n", p=128))
            adjT_t = cpool.tile([128, KH, RP], F32)
            nc.scalar.dma_start(adjT_t[:],
                                adjT_d.rearrange("(k p) m -> p k m", p=128))
            adj_t = cpool.tile([128, N], F32)
            nc.scalar.dma_start(adj_t[:], adj_d[:])

            # ---- xW = x @ W : [N, N] in SBUF as 8 k-tiles of [128, N] ----
            xw_sb = wpool.tile([128, KH, N], F32)
            for m in range(KH):
                for n in range(NT):
                    ps = psA.tile([128, 512], F32, tag="ps_xw")
                    for k in range(KX):
                        nc.tensor.matmul(
                            ps[:],
                            xT_t[:, k, m * 128:(m + 1) * 128],
                            W_t[:, k, n * 512:(n + 1) * 512],
                            start=(k == 0), stop=(k == KX - 1),
                        )
                    nc.scalar.copy(xw_sb[:, m, n * 512:(n + 1) * 512], ps[:])

            # ---- h = adj_shard @ xW : [128, N] ----
            h_sb = wpool.tile([128, N], F32)
            xprob = wpool.tile([128, N], F32)
            for n in range(NT):
                ph = psB.tile([128, 512], F32, tag="ps_h")
                for k in range(KH):
                    nc.tensor.matmul(
                        ph[:],
                        adjT_t[:, k, :],
                        xw_sb[:, k, n * 512:(n + 1) * 512],
                        start=(k == 0), stop=(k == KH - 1),
                    )
                nc.scalar.copy(h_sb[:, n * 512:(n + 1) * 512], ph[:])
                nc.scalar.activation(xprob[:, n * 512:(n + 1) * 512], ph[:],
                                     AF.Sigmoid)
            nc.scalar.dma_start(h_d[:], h_sb[:])

            # ---- info_loss partials: sum_j (xprob - thr)^2 per row ----
            sq = wpool.tile([128, N], F32, tag="scratch")
            loss_rows = wpool.tile([128, 1], F32)
            nthr_b = wpool.tile([128, 1], F32)
            nc.gpsimd.memset(nthr_b[:], -float(threshold))
            nc.scalar.activation(sq[:], xprob[:], AF.Square,
                                 bias=nthr_b[:], accum_out=loss_rows[:])
            nc.scalar.dma_start(lr_d[:], loss_rows[:])

            # ---- V4 += noise chunks (streaming accumulation) ----
            # V4 is a [128, 4, N] fp32 accumulator; each 12-sample fp16
            # chunk is added as three [128, 4, N] tensor_adds (fp16 read,
            # fp32 accumulate). The final chunk's adds and the whole tail
            # are split by column halves so the tail pipeline starts before
            # the last full-width op finishes.
            nc.vector.tensor_add(V4[:], V4[:], nz0[:])
            off = CHUNKS[0]
            for ci, ch in enumerate(CHUNKS[1:]):
                last = ci == len(CHUNKS) - 2
                nz = npool.tile([128, ch, N], F16, tag="noise_chunk")
                nc.sync.dma_start(nz[:], noise_d[:, off:off + ch, :])
                for s in range(0, ch, 4):
                    if not last:
                        nc.vector.tensor_add(V4[:], V4[:], nz[:, s:s + 4, :])
                    else:
                        for hc in range(2):
                            cs = slice(hc * (N // 2), (hc + 1) * (N // 2))
                            nc.vector.tensor_add(V4[:, :, cs], V4[:, :, cs],
                                                 nz[:, s:s + 4, cs])
                off += ch

            # ---- x_sample = sigmoid((V/S + xprob - 1)/tau); adj_new ----
            t1 = wpool.tile([128, N], F32, tag="scratch")
            xs = wpool.tile([128, N], F32)
            ntau_b = wpool.tile([128, 1], F32)
            nc.gpsimd.memset(ntau_b[:], -1.0 / float(tau))
            for hc in range(2):
                cs = slice(hc * (N // 2), (hc + 1) * (N // 2))
                # fold V4 -> V for this column half: [4]->[2]->[1]
                nc.vector.tensor_add(V4[:, 0:2, cs], V4[:, 0:2, cs],
                                     V4[:, 2:4, cs])
                nc.vector.tensor_add(V4[:, 0, cs], V4[:, 0, cs], V4[:, 1, cs])
                nc.vector.scalar_tensor_tensor(t1[:, cs], V4[:, 0, cs],
                                               1.0 / S, xprob[:, cs],
                                               op0=ALU.mult, op1=ALU.add)
                nc.scalar.activation(xs[:, cs], t1[:, cs], AF.Sigmoid,
                                     scale=1.0 / float(tau),
                                     bias=ntau_b[:])
                # adj_new in place over adj_t, then store
                nc.vector.tensor_mul(adj_t[:, cs], adj_t[:, cs], xs[:, cs])
                nc.scalar.dma_start(an_d[:, cs], adj_t[:, cs])

    nc.compile()
    return nc


_PROGRAM_CACHE = {}


def _get_program(tau: float, threshold: float):
    key = (tau, threshold)
    if key not in _PROGRAM_CACHE:
        _PROGRAM_CACHE[key] = build_program(tau, threshold)
    return _PROGRAM_CACHE[key]


def make_in_maps(x, W, adj, noise):
    xT = np.ascontiguousarray(np.asarray(x, np.float32).T)
    Wc = np.ascontiguousarray(np.asarray(W, np.float32))
    adj_c = np.ascontiguousarray(np.asarray(adj, np.float32))
    noise_h = np.asarray(noise, np.float16)
    in_maps = []
    for i in range(NCORES):
        r0 = i * RP
        in_maps.append({
            "xT": xT,
            "W": Wc,
            "adj": adj_c[r0:r0 + RP],
            "adjT": np.ascontiguousarray(adj_c[r0:r0 + RP].T),
            "noise": noise_h[r0:r0 + RP],
        })
    return in_maps


def assemble(results):
    h = np.concatenate([results[i]["h_out"] for i in range(NCORES)], axis=0)
    adj_new = np.concatenate([results[i]["adjnew_out"] for i in range(NCORES)],
                             axis=0)
    total = 0.0
    for i in range(NCORES):
        total += float(results[i]["loss_out"].astype(np.float64).sum())
    info_loss = np.float32(0.5 * total)
    return h, adj_new, info_loss


def kernel(x, W, adj, noise, tau, threshold):
    tau_f = float(np.asarray(tau))
    thr_f = float(np.asarray(threshold))
    nc = _get_program(tau_f, thr_f)
    in_maps = make_in_maps(x, W, adj, noise)
    res = run_bass_kernel_spmd(nc, in_maps, core_ids=list(range(NCORES)))
    return assemble(res.results)


# revision 45
# speedup vs baseline: 1.1357x; 1.1357x over previous
"""Trainium2 Bass kernel for nn_Adj_adjust — DVE/PE hybrid V-reduction.

Same contract as kernel.py (v7). Differences:
  - V = noise.mean(axis=1) split: columns [0, CD) accumulated on the
    VectorEngine from a row-major fp16 stream; columns [CD, N) reduced on
    the TensorEngine via per-column (LDWEIGHTS + N=1 MATMUL) against a
    ones vector, from a [sample(padded to 128), col, row] fp16 layout
    (padding keeps the DMA spread over all 16 SDMA engines).
  - TensorEngine stream order: V cols (chunks 0-1) -> xW -> V cols
    (chunks 2..) -> h, with PE-chunk DMA triggers threaded through the
    xW copy loop on the ACT sequencer.
"""

import numpy as np

import concourse.bacc as bacc
import concourse.mybir as mybir
from concourse import tile
from concourse.bass_utils import run_bass_kernel_spmd

N = 1024
D = 256
S = 100
NCORES = 8
RP = N // NCORES

F32 = mybir.dt.float32
F16 = mybir.dt.float16
AF = mybir.ActivationFunctionType
ALU = mybir.AluOpType

CD = 576              # columns reduced on DVE; the rest go to PE
CP = N - CD           # 448 PE columns (fits one PSUM bank)
SCHUNKS = [10] * 10               # DVE stream: samples per chunk
PCHUNKS = [64] * 7                # PE stream: columns per chunk
assert sum(SCHUNKS) == S and sum(PCHUNKS) == CP


def build_program(tau: float, threshold: float):
    nc = bacc.Bacc("TRN2", target_bir_lowering=False, debug=False,
                   num_devices=NCORES)

    xT_d = nc.dram_tensor("xT", [D, N], F32, kind="ExternalInput")
    W_d = nc.dram_tensor("W", [D, N], F32, kind="ExternalInput")
    adj_d = nc.dram_tensor("adj", [RP, N], F32, kind="ExternalInput")
    adjT_d = nc.dram_tensor("adjT", [N, RP], F32, kind="ExternalInput")
    nd_d = nc.dram_tensor("noise_dve", [RP, S, CD], F16, kind="ExternalInput")
    np_d = nc.dram_tensor("noise_pe", [128, CP, RP], F16, kind="ExternalInput")

    h_d = nc.dram_tensor("h_out", [RP, N], F32, kind="ExternalOutput")
    an_d = nc.dram_tensor("adjnew_out", [RP, N], F32, kind="ExternalOutput")
    lr_d = nc.dram_tensor("loss_out", [RP, 1], F32, kind="ExternalOutput")

    KX = D // 128
    KH = N // 128
    NT = N // 512

    with tile.TileContext(nc) as tc:
        with (
            tc.tile_pool(name="const", bufs=1) as cpool,
            tc.tile_pool(name="noised", bufs=5) as ndpool,
            tc.tile_pool(name="noisep", bufs=3) as nppool,
            tc.tile_pool(name="work", bufs=1) as wpool,
            tc.tile_pool(name="psA", bufs=4, space="PSUM") as psA,
            tc.tile_pool(name="psB", bufs=2, space="PSUM") as psB,
            tc.tile_pool(name="psV", bufs=1, space="PSUM") as psV,
        ):
            # the Sync ring carries ONLY the DVE noise stream
            V4 = wpool.tile([128, 4, CD], F32)
            nz0 = ndpool.tile([128, SCHUNKS[0], CD], F16, tag="nd_chunk",
                              name="nd0")
            nc.sync.dma_start(nz0[:], nd_d[:, 0:SCHUNKS[0], :])

            # GEMM operands head the Scalar ring, PE noise follows
            xT_t = cpool.tile([128, KX, N], F32)
            nc.scalar.dma_start(xT_t[:], xT_d.rearrange("(k p) n -> p k n", p=128))
            W_t = cpool.tile([128, KX, N], F32)
            nc.scalar.dma_start(W_t[:], W_d.rearrange("(k p) n -> p k n", p=128))
            adjT_t = cpool.tile([128, KH, RP], F32)
            nc.scalar.dma_start(adjT_t[:],
                                adjT_d.rearrange("(k p) m -> p k m", p=128))
            adj_t = cpool.tile([128, N], F32)
            nc.scalar.dma_start(adj_t[:], adj_d[:])

            ones_t = cpool.tile([128, 1], F16)
            nc.gpsimd.memset(ones_t[:], 1.0)

            dve_tiles = [(nz0, 0, SCHUNKS[0])]
            off = SCHUNKS[0]
            for ch in SCHUNKS[1:]:
                nz = ndpool.tile([128, ch, CD], F16, tag="nd_chunk",
                                 name=f"nd{off}")
                nc.sync.dma_start(nz[:], nd_d[:, off:off + ch, :])
                dve_tiles.append((nz, off, ch))
                off += ch
            # last two PE chunks ride the Sync ring after the DVE stream
            # (their tile slots free only late; blocking Sync is harmless)

            # PE noise stream on the Scalar ring: chunks 0-1 up front,
            # the rest threaded into the xW copy loop
            pe_tiles = []
            c0 = 0
            for cc in PCHUNKS:
                nzp = nppool.tile([128, cc, RP], F16, tag="np_chunk",
                                  name=f"np{c0}")
                pe_tiles.append((nzp, c0, cc))
                c0 += cc
            for nzp, c0, cc in pe_tiles[0:2]:
                nc.scalar.dma_start(nzp[:], np_d[:, c0:c0 + cc, :])

            # DVE accumulation (paced by the sync-ring stream); first op
            # initializes V4 by copy so no memset is needed
            nc.vector.tensor_copy(V4[:], nz0[:, 0:4, :])
            for i, (nz, off, ch) in enumerate(dve_tiles):
                last = i == len(dve_tiles) - 1
                s0 = 4 if i == 0 else 0
                for s in range(s0, ch, 4):
                    w = min(4, ch - s)
                    if not last:
                        nc.vector.tensor_add(V4[:, 0:w, :], V4[:, 0:w, :],
                                             nz[:, s:s + w, :])
                    else:
                        for hc in range(2):
                            cs = slice(hc * (CD // 2), (hc + 1) * (CD // 2))
                            nc.vector.tensor_add(V4[:, 0:w, cs], V4[:, 0:w, cs],
                                                 nz[:, s:s + w, cs])

            # TensorEngine stream: V cols (0-1) -> xW -> V cols (2..) -> h
            psv = psV.tile([128, CP], F32)
            v_pe = wpool.tile([128, CP], F32)

            def v_mms(chunk):
                nzp, c0, cc = chunk
                for j in range(cc):
                    nc.tensor.matmul(psv[:, c0 + j:c0 + j + 1], nzp[:, j, :],
                                     ones_t[:], start=True, stop=True)

            for chunk in pe_tiles[0:2]:
                v_mms(chunk)

            xw_sb = wpool.tile([128, KH, N], F32)
            pe_next = 2
            for m in range(KH):
                for n in range(NT):
                    ps = psA.tile([128, 512], F32, tag="ps_xw")
                    for k in range(KX):
                        nc.tensor.matmul(
                            ps[:],
                            xT_t[:, k, m * 128:(m + 1) * 128],
                            W_t[:, k, n * 512:(n + 1) * 512],
                            start=(k == 0), stop=(k == KX - 1),
                        )
                    nc.scalar.copy(xw_sb[:, m, n * 512:(n + 1) * 512], ps[:])
                    if (m * NT + n) in (4, 7, 10, 13) and pe_next < 6:
                        nzp, c0, cc = pe_tiles[pe_next]
                        nc.scalar.dma_start(nzp[:], np_d[:, c0:c0 + cc, :])
                        pe_next += 1
            nzp, c0, cc = pe_tiles[6]
            nc.scalar.dma_start(nzp[:], np_d[:, c0:c0 + cc, :])

            for chunk in pe_tiles[2:]:
                v_mms(chunk)
            nc.scalar.copy(v_pe[:], psv[:])

            # h = adj_shard @ xW
            h_sb = wpool.tile([128, N], F32)
            xprob = wpool.tile([128, N], F32)
            for n in range(NT):
                ph = psB.tile([128, 512], F32, tag="ps_h")
                for k in range(KH):
                    nc.tensor.matmul(
                        ph[:],
                        adjT_t[:, k, :],
                        xw_sb[:, k, n * 512:(n + 1) * 512],
                        start=(k == 0), stop=(k == KH - 1),
                    )
                nc.scalar.copy(h_sb[:, n * 512:(n + 1) * 512], ph[:])
                nc.scalar.activation(xprob[:, n * 512:(n + 1) * 512], ph[:],
                                     AF.Sigmoid)
            nc.scalar.dma_start(h_d[:], h_sb[:])

            # info_loss partials
            sq = wpool.tile([128, N], F32, tag="scratch")
            loss_rows = wpool.tile([128, 1], F32)
            nthr_b = wpool.tile([128, 1], F32)
            nc.gpsimd.memset(nthr_b[:], -float(threshold))
            nc.scalar.activation(sq[:], xprob[:], AF.Square,
                                 bias=nthr_b[:], accum_out=loss_rows[:])
            nc.scalar.dma_start(lr_d[:], loss_rows[:])

            # fold V4; x_sample; adj_new
            t1 = wpool.tile([128, N], F32, tag="scratch")
            xs = wpool.tile([128, N], F32)
            ntau_b = wpool.tile([128, 1], F32)
            nc.gpsimd.memset(ntau_b[:], -1.0 / float(tau))

            for hc in range(2):
                cs = slice(hc * (CD // 2), (hc + 1) * (CD // 2))
                nc.vector.tensor_add(V4[:, 0:2, cs], V4[:, 0:2, cs],
                                     V4[:, 2:4, cs])
                nc.vector.tensor_add(V4[:, 0, cs], V4[:, 0, cs], V4[:, 1, cs])
                nc.vector.scalar_tensor_tensor(t1[:, cs], V4[:, 0, cs],
                                               1.0 / S, xprob[:, cs],
                                               op0=ALU.mult, op1=ALU.add)
                nc.scalar.activation(xs[:, cs], t1[:, cs], AF.Sigmoid,
                                     scale=1.0 / float(tau), bias=ntau_b[:])
                nc.vector.tensor_mul(adj_t[:, cs], adj_t[:, cs], xs[:, cs])
                nc.scalar.dma_start(an_d[:, cs], adj_t[:, cs])
            csp = slice(CD, N)
            nc.vector.scalar_tensor_tensor(t1[:, csp], v_pe[:], 1.0 / S,
                                           xprob[:, csp],
                                           op0=ALU.mult, op1=ALU.add)
            nc.scalar.activation(xs[:, csp], t1[:, csp], AF.Sigmoid,
                                 scale=1.0 / float(tau), bias=ntau_b[:])
            nc.vector.tensor_mul(adj_t[:, csp], adj_t[:, csp], xs[:, csp])
            nc.scalar.dma_start(an_d[:, csp], adj_t[:, csp])

    nc.compile()
    return nc


_PROGRAM_CACHE = {}


def _get_program(tau: float, threshold: float):
    key = (tau, threshold)
    if key not in _PROGRAM_CACHE:
        _PROGRAM_CACHE[key] = build_program(tau, threshold)
    return _PROGRAM_CACHE[key]


def make_in_maps(x, W, adj, noise):
    xT = np.ascontiguousarray(np.asarray(x, np.float32).T)
    Wc = np.ascontiguousarray(np.asarray(W, np.float32))
    adj_c = np.ascontiguousarray(np.asarray(adj, np.float32))
    noise_h = np.asarray(noise, np.float16)
    in_maps = []
    for i in range(NCORES):
        r0 = i * RP
        shard = noise_h[r0:r0 + RP]
        npe = np.zeros((128, CP, RP), np.float16)
        npe[:S] = shard[:, :, CD:].transpose(1, 2, 0)
        in_maps.append({
            "xT": xT,
            "W": Wc,
            "adj": adj_c[r0:r0 + RP],
            "adjT": np.ascontiguousarray(adj_c[r0:r0 + RP].T),
            "noise_dve": np.ascontiguousarray(shard[:, :, 0:CD]),
            "noise_pe": npe,
        })
    return in_maps


def assemble(results):
    h = np.concatenate([results[i]["h_out"] for i in range(NCORES)], axis=0)
    adj_new = np.concatenate([results[i]["adjnew_out"] for i in range(NCORES)],
                             axis=0)
    total = 0.0
    for i in range(NCORES):
        total += float(results[i]["loss_out"].astype(np.float64).sum())
    info_loss = np.float32(0.5 * total)
    return h, adj_new, info_loss


def kernel(x, W, adj, noise, tau, threshold):
    tau_f = float(np.asarray(tau))
    thr_f = float(np.asarray(threshold))
    nc = _get_program(tau_f, thr_f)
    in_maps = make_in_maps(x, W, adj, noise)
    res = run_bass_kernel_spmd(nc, in_maps, core_ids=list(range(NCORES)))
    return assemble(res.results)
